# revision 19
# baseline (speedup 1.0000x reference)
"""CQC contrastive loss kernel for 8 Trainium2 NeuronCores.

Math (B=4096, D=256, TAU=0.5, N=2B=8192):
    x  = concat(Xa, Za)                      [N, D]
    xn = x / ||x||                           (row-normalized)
    S  = xn @ xn.T                           [N, N]
    loss_i = log(sum_{j != i} exp(S_ij/TAU)) - S[i, i+-B]/TAU
    loss   = mean_i loss_i

Split of work (wall time of a warm call is dominated by the axon tunnel:
tens-of-ms round trips, ~70 MB/s host->device, so the design minimizes
bytes moved and round trips, not device cycles):

  Host: per 2048-row chunk, quantize rows to int4 with a per-row scale
      (q_i = round(x_i * 7 / max|x_i|), scale s_i = max|x_i| / (7 ||x_i||);
      simulated end-to-end rel err 2.8e-5) and pack two nibbles per byte
      via a small XLA-cpu jit; each chunk's async sharded device_put
      streams while the next chunk is computed. Only ~1 MB crosses the
      tunnel. The positive-pair dot sum pos_i = xn_i . xn_{i+-B} is
      computed on the host in f32 and overlaps the upload tail. The f32
      per-row scales (32 KB) are uploaded once, pre-permuted into per-core
      slab order.
  Device (per core): AllGather the packed slabs and the scales over
      NeuronLink (rank order; the row-sum over all columns is
      permutation-invariant so gather order never matters), unpack nibbles
      (DVE bitwise_and / shift, then one casting (q-8)*s tensor_scalar into
      bf16), PE-transpose into column-major xnT, bf16 matmuls of the
      own-slab block against all N columns accumulating S in PSUM, ScalarE
      exp(2*S) with fused row-sum, then
      lg_i = log(rowsum_i - exp(2*||xn_i||^2)), reduce the 8 row blocks and
      DMA out [128, 1] per core.
  Host: loss = (sum_i lg_i - 2 * sum_i pos_i) / N.

The jitted executable, the Bass module, and the compiled NEFF are cached at
module level: warm calls pay only host math, the ~1 MB upload, and one
execute round trip (the tiny output rides back with the completion).
"""

import numpy as np
import ml_dtypes

import jax
import jax.numpy as jnp
from jax.sharding import Mesh, NamedSharding, PartitionSpec

try:
    from jax.experimental.shard_map import shard_map
except ImportError:  # newer jax
    from jax import shard_map

import concourse.bacc as bacc
import concourse.tile as tile
from concourse import mybir
from concourse import bass2jax

F32 = mybir.dt.float32
BF16 = mybir.dt.bfloat16
U8 = mybir.dt.uint8
AL = mybir.AluOpType
AF = mybir.ActivationFunctionType

B = 4096
D = 256
N = 2 * B
TAU = 0.5
NCORES = 8
RPC = N // NCORES          # rows per core = 1024
NBLK = RPC // 128          # 128-row blocks per core = 8
NT = N // 128              # 128-row tiles in the gathered x = 64
GRP = 8                    # unpack/transpose phases (8 tiles each)
TPG = NT // GRP            # tiles per phase = 8
NCHUNK = 4                 # host->device upload pipeline chunks
CROWS = N // NCHUNK        # global rows per chunk = 2048
CPC = RPC // NCHUNK        # chunk rows per core = 256
DP = D // 2                # packed bytes per row = 128
QMAX = 7                   # int4 symmetric range [-7, 7], stored offset +8
# main-loop chunk groups (in 512-col units): 16 chunks -> 6 groups sized to
# fit a 3-bank [128, 1536] f32 PSUM tile
CGS = [(0, 1, 2), (3, 4, 5), (6, 7, 8), (9, 10, 11), (12, 13, 14), (15,)]
NCG = len(CGS)

# scales upload permutation: core c's input rows are chunk-striped; see
# kernel(). PERM[1024c + 256k + j] = 2048k + 256c + j
_PERM = np.empty(N, np.int64)
for _c in range(NCORES):
    for _k in range(NCHUNK):
        _j = np.arange(CPC)
        _PERM[RPC * _c + CPC * _k + _j] = CROWS * _k + CPC * _c + _j


def _patch_act_tables():
    """Force every activation onto the one table set that covers both exp
    and ln, so the kernel pays a single ACT table load instead of two.
    Indices of the other sets are kept (emptied, not removed) because
    act_func_set_id is a positional index into act_info.json."""
    if getattr(bacc, "_cqc_act_patch", False):
        return
    orig = bacc.get_activation_tables

    def patched(module_arch):
        tabs = orig(module_arch)
        keep = "natural_log_exp_and_others"
        if keep in tabs:
            tabs = {name: (fns if name == keep else set())
                    for name, fns in tabs.items()}
        return tabs

    bacc.get_activation_tables = patched
    bacc._cqc_act_patch = True


def build():
    _patch_act_tables()
    nc = bacc.Bacc("TRN2", target_bir_lowering=False, debug=False,
                   num_devices=NCORES)

    Pcs = [nc.dram_tensor(f"P{k}", [CPC, DP], U8, kind="ExternalInput").ap()
           for k in range(NCHUNK)]
    SC = nc.dram_tensor("SC", [RPC, 1], F32, kind="ExternalInput").ap()
    oLoss = nc.dram_tensor("loss", [128, 1], F32,
                           kind="ExternalOutput").ap()
    ident = nc.inline_tensor(np.eye(128, dtype=ml_dtypes.bfloat16),
                             name="ident").ap()

    with tile.TileContext(nc) as tc:
        with (
            tc.tile_pool(name="dram", bufs=1, space="DRAM") as dr,
            tc.tile_pool(name="stream", bufs=3) as st,
            tc.tile_pool(name="persist", bufs=1) as pr,
            tc.tile_pool(name="psum", bufs=2, space="PSUM") as ps,
        ):
            # --- AllGather packed slabs + scales (bounce via internal DRAM).
            # The slab arrives as NCHUNK pipelined upload chunks; their
            # concatenation (and hence the gathered row order) is a fixed
            # permutation of the global rows, which is harmless: the row-sum
            # runs over all columns and the host only consumes the SUM of
            # the per-row losses. Scales are host-permuted to match. ---
            inb = dr.tile([RPC, DP], U8)
            for k in range(NCHUNK):
                nc.gpsimd.dma_start(inb[k * CPC:(k + 1) * CPC, :], Pcs[k])
            inb_s = dr.tile([RPC, 1], F32)
            nc.gpsimd.dma_start(inb_s, SC)
            gxp = dr.tile([N, DP], U8, addr_space="Shared")
            nc.gpsimd.collective_compute(
                "AllGather", AL.bypass,
                replica_groups=[list(range(NCORES))],
                ins=[inb], outs=[gxp])
            gxs = dr.tile([N, 1], F32, addr_space="Shared")
            nc.gpsimd.collective_compute(
                "AllGather", AL.bypass,
                replica_groups=[list(range(NCORES))],
                ins=[inb_s], outs=[gxs])
            gxt = gxp.rearrange("(t p) d -> p t d", p=128)   # [128, 64, 128]
            inbt = inb.rearrange("(t p) d -> p t d", p=128)  # [128, 8, 128]

            idt = pr.tile([128, 128], BF16, tag="ident")
            nc.sync.dma_start(out=idt, in_=ident)
            # scales: [p, t] = scale of gathered row 128t + p
            sct = pr.tile([128, NT], F32, tag="sct")
            nc.sync.dma_start(out=sct,
                              in_=gxs.rearrange("(t p) o -> p (t o)", p=128))
            sco = pr.tile([128, NBLK], F32, tag="sco")
            nc.sync.dma_start(out=sco,
                              in_=inb_s.rearrange("(t p) o -> p (t o)", p=128))

            sdiag = pr.tile([128, NBLK], F32, tag="sdiag")
            rs_parts = pr.tile([128, NBLK * NCG], F32, tag="rsp")

            # xnT[k][g]: [128, 1024] bf16 -- d-half k, 1024-col group g
            xnT = [[pr.tile([128, TPG * 128], BF16, tag=f"xnT{k}_{g}",
                            name=f"xnT{k}_{g}")
                    for g in range(GRP)] for k in range(2)]
            # lhsT[k]: [128, 1024] bf16 -- transposed own slab, block b at
            # cols [128b, 128b+128)
            lhsT = [pr.tile([128, RPC], BF16, tag=f"lhsT{k}",
                            name=f"lhsT{k}") for k in range(2)]

            def unpack_tiles(src, ntiles, scales, xb):
                """src [128, ntiles, 128] u8 -> xb [128, ntiles, 256] bf16,
                dequantized with per-row scales[:, t]."""
                for t in range(ntiles):
                    nib = st.tile([128, DP, 2], U8, tag="nib", name="nib")
                    nc.vector.tensor_scalar(
                        out=nib[:, :, 0], in0=src[:, t, :], scalar1=0x0F,
                        scalar2=None, op0=AL.bitwise_and)
                    nc.vector.tensor_scalar(
                        out=nib[:, :, 1], in0=src[:, t, :], scalar1=4,
                        scalar2=None, op0=AL.logical_shift_right)
                    nc.vector.tensor_scalar(
                        out=xb[:, t, :], in0=nib.rearrange("p a b -> p (a b)"),
                        scalar1=-8.0, scalar2=scales[:, t:t + 1],
                        op0=AL.add, op1=AL.mult)

            def own_slab():
                xs = pr.tile([128, NBLK, DP], U8, tag="xs")
                nc.sync.dma_start(out=xs, in_=inbt)
                xb = pr.tile([128, NBLK, D], BF16, tag="xbo")
                unpack_tiles(xs, NBLK, sco, xb)
                for t in range(NBLK):
                    scr = st.tile([128, D], BF16, tag="sq", name="sq")
                    nc.vector.scalar_tensor_tensor(
                        out=scr, in0=xb[:, t, :], scalar=1.0, in1=xb[:, t, :],
                        op0=AL.mult, op1=AL.mult,
                        accum_out=sdiag[:, t:t + 1])
                for k in range(2):
                    pt = ps.tile([128, NBLK * 128], BF16, tag="tp", name="pt")
                    for t in range(NBLK):
                        nc.tensor.transpose(
                            pt[:, t * 128:(t + 1) * 128],
                            xb[:, t, k * 128:(k + 1) * 128], idt)
                    nc.vector.tensor_copy(lhsT[k], pt)

            def phase0(g):
                xg = st.tile([128, TPG, DP], U8, tag="xg", name="xg")
                nc.sync.dma_start(out=xg, in_=gxt[:, g * TPG:(g + 1) * TPG, :])
                xb = st.tile([128, TPG, D], BF16, tag="xb", name="xb")
                unpack_tiles(xg, TPG, sct[:, g * TPG:(g + 1) * TPG], xb)
                for k in range(2):
                    pt = ps.tile([128, TPG * 128], BF16, tag="tp", name="pt")
                    for t in range(TPG):
                        nc.tensor.transpose(
                            pt[:, t * 128:(t + 1) * 128],
                            xb[:, t, k * 128:(k + 1) * 128], idt)
                    nc.vector.tensor_copy(xnT[k][g], pt)

            def main_cg(b, cgi):
                cg = CGS[cgi]
                w = len(cg) * 512
                pm = ps.tile([128, w], F32, tag="big", name="pm",
                             padded_shape=[128, 3 * 512])
                for k in range(2):
                    lh = lhsT[k][:, b * 128:(b + 1) * 128]
                    for i, c in enumerate(cg):
                        nc.tensor.matmul(
                            pm[:, i * 512:(i + 1) * 512], lh,
                            xnT[k][c // 2]
                               [:, (c % 2) * 512:(c % 2 + 1) * 512],
                            start=(k == 0), stop=(k == 1))
                escr = st.tile([128, w], BF16, tag="exps", name="exps",
                               padded_shape=[128, 3 * 512])
                col = b * NCG + cgi
                nc.scalar.activation(
                    out=escr, in_=pm, func=AF.Exp, scale=2.0,
                    accum_out=rs_parts[:, col:col + 1])

            own_slab()
            for g in range(GRP):
                phase0(g)
            for b in range(NBLK):
                for cgi in range(NCG):
                    main_cg(b, cgi)

            # --- finals: lg = log(rowsum - exp(2*sdiag)), reduce blocks ---
            rs_tot = pr.tile([128, NBLK], F32, tag="rs_tot")
            nc.vector.tensor_reduce(
                out=rs_tot,
                in_=rs_parts.rearrange("p (b g) -> p b g", g=NCG),
                op=AL.add, axis=mybir.AxisListType.X)
            e_diag = pr.tile([128, NBLK], F32, tag="e_diag")
            nc.scalar.activation(out=e_diag, in_=sdiag, func=AF.Exp,
                                 scale=2.0)
            rsm = pr.tile([128, NBLK], F32, tag="rsm")
            nc.vector.tensor_sub(rsm, rs_tot, e_diag)
            lg = pr.tile([128, NBLK], F32, tag="lg")
            nc.scalar.activation(out=lg, in_=rsm, func=AF.Ln)
            lgs = pr.tile([128, 1], F32, tag="lgs")
            nc.vector.tensor_reduce(out=lgs, in_=lg, op=AL.add,
                                    axis=mybir.AxisListType.X)
            nc.sync.dma_start(out=oLoss, in_=lgs)

    nc.finalize()
    return nc


_CACHE = {}
last_results = None


@jax.jit
def _quant_pack(Xk, nrmk):
    # int4 per-row quantize + nibble pack; runs on CPU (inputs committed
    # there). Returns packed [CROWS, DP] u8 and dequant scales [CROWS] f32.
    am = jnp.max(jnp.abs(Xk), axis=1)
    am = jnp.maximum(am, 1e-30)
    qi = jnp.clip(jnp.round(Xk * (QMAX / am)[:, None]), -QMAX, QMAX)
    qu = (qi + 8.0).astype(jnp.uint8)
    packed = qu[:, 0::2] | (qu[:, 1::2] << 4)
    return packed, am / (QMAX * nrmk)


def _setup():
    nc = build()
    bass2jax.install_neuronx_cc_hook()

    partition_name = (nc.partition_id_tensor.name
                      if nc.partition_id_tensor else None)
    in_names, out_names, out_avals = [], [], []
    for alloc in nc.m.functions[0].allocations:
        if not isinstance(alloc, mybir.MemoryLocationSet):
            continue
        name = alloc.memorylocations[0].name
        if alloc.kind == "ExternalInput":
            if name != partition_name:
                in_names.append(name)
        elif alloc.kind == "ExternalOutput":
            out_names.append(name)
            out_avals.append(jax.core.ShapedArray(
                tuple(alloc.tensor_shape), mybir.dt.np(alloc.dtype)))
    assert in_names == [f"P{k}" for k in range(NCHUNK)] + ["SC"], in_names
    assert out_names == ["loss"], out_names
    n_params = len(in_names)
    n_outs = len(out_avals)
    # No donated zero output buffers: the kernel writes every element of
    # "loss", and the neuronx hook renames it to output0 anyway (out_rename
    # wins the dict union), so a donated operand would bind to nothing.
    in_names_full = in_names + ([partition_name] if partition_name else [])

    def _body(*args):
        operands = list(args)
        if partition_name is not None:
            operands.append(bass2jax.partition_id_tensor())
        outs = bass2jax._bass_exec_p.bind(
            *operands, out_avals=tuple(out_avals),
            in_names=tuple(in_names_full), out_names=tuple(out_names),
            lowering_input_output_aliases=(),
            sim_require_finite=True, sim_require_nnan=True, nc=nc)
        return tuple(outs)

    devices = jax.devices()[:NCORES]
    assert len(devices) == NCORES, (
        f"need {NCORES} devices, found {len(jax.devices())}")
    mesh = Mesh(np.asarray(devices), ("core",))
    sharded = jax.jit(
        shard_map(_body, mesh=mesh,
                  in_specs=(PartitionSpec("core"),) * n_params,
                  out_specs=(PartitionSpec("core"),) * n_outs,
                  check_rep=False),
        keep_unused=True)
    _CACHE["fn"] = sharded
    _CACHE["sharding"] = NamedSharding(mesh, PartitionSpec("core"))


def kernel(Xa: np.ndarray, Za: np.ndarray) -> np.ndarray:
    if "fn" not in _CACHE:
        _setup()
    fn = _CACHE["fn"]
    sh = _CACHE["sharding"]
    cpu = jax.devices("cpu")[0]

    # --- host: per-chunk int4 quantize+pack (XLA cpu); each chunk's async
    # sharded device_put streams while the next chunk is computed ---
    Xa = np.asarray(Xa)
    Za = np.asarray(Za)
    X = np.empty((N, D), np.float32)
    nrm = np.empty((N,), np.float32)
    scales = np.empty((N,), np.float32)
    dchunks = []
    for k in range(NCHUNK):
        lo = k * CROWS
        src = Xa if lo < B else Za
        s0 = lo % B
        Xk = X[lo:lo + CROWS]
        Xk[:] = src[s0:s0 + CROWS]
        nk = np.maximum(np.sqrt(np.einsum("ij,ij->i", Xk, Xk)), 1e-8)
        nrm[lo:lo + CROWS] = nk
        pk, sk = _quant_pack(jax.device_put(Xk, cpu), jax.device_put(nk, cpu))
        scales[lo:lo + CROWS] = np.asarray(sk)
        dchunks.append(jax.device_put(pk, sh))   # async upload

    # scales, permuted into per-core slab order (tiny, one put)
    dsc = jax.device_put(
        np.ascontiguousarray(scales[_PERM]).reshape(N, 1), sh)

    # pos on raw rows (overlaps the tail of the uploads):
    # pos_i = (x_i . x_{i+B}) / (|x_i| |x_{i+B}|)
    pd = np.einsum("ij,ij->i", X[:B], X[B:])
    p0sum = float((pd / (nrm[:B] * nrm[B:])).sum(dtype=np.float64))

    out = fn(*dchunks, dsc)                      # async dispatch to trn2
    lg = np.asarray(out[0])                      # [8*128, 1]

    loss = (lg.astype(np.float64).sum() - 4.0 * p0sum) / N
    return np.float32(loss)
